# revision 30
# baseline (speedup 1.0000x reference)
"""ECE loss kernel for Trainium2 (Bass/Tile), data-parallel over 8 NeuronCores.

Math (per sample row of logits[N, C]):
  conf = max softmax(x) = exp(max(x)) / sum(exp(x))
  acc  = (argmax(x) == label)
  ece  = sum_b |conf_sum[b] - acc_sum[b]| / N     (15 bins + empty tail)

Device pipeline per core (125184 rows as [128 partitions x 978 samples],
tiles of 16 samples x 100 classes):
  - DMA x tiles on the two HWDGE rings (sync/scalar)
  - ACT: E = exp(x) in place
  - Pool (GpSimd): two max tree-fold levels over E (100 -> 50 -> 25) —
    the otherwise-idle engine absorbs 3/4 of the row-max work; max-folds
    are numerically exact in any order
  - DVE: reduce_max over the 25-wide folds + reduce_sum over E (the only
    irreducible 1x pass), then conf/acc and 49 tensor_scalar+accum passes
    for the per-bin cumulative sums (wt/nn/av at 2x fp32) + total accuracy
Host: gathers g = x[i,label_i] (1% of bytes), pads 1472 zero rows, sums the
per-core [128, 49] outputs and finishes the tiny 16-bin reduction.
"""

import os

import numpy as np

import concourse.bass as bass
import concourse.mybir as mybir
import concourse.tile as tile
from concourse.bass_utils import run_bass_kernel_spmd

F32 = mybir.dt.float32
ALU = mybir.AluOpType
AX = mybir.AxisListType
ACTF = mybir.ActivationFunctionType

N = 1_000_000
C = 100
NCORES = 8
P = 128
SPP = 978                   # samples per partition (padded)
ROWS = P * SPP              # 125184 rows per core
NTOT = NCORES * ROWS        # 1001472
PAD = NTOT - N              # 1472 zero pad rows (conf=0.01, acc=1)
K = 64                      # samples per tile
SIZES = [K] * 15 + [18]     # sum = 978
CHUNKS = [(0, 512), (512, 960), (960, 978)]  # binning chunks (tile-aligned)
N_ACT_CHUNKS = 2            # first chunks bin on ACT (overlap the main loop);
                            # the small tail chunk bins on the then-idle DVE
NBINS = 16

LAST_RESULTS = None


def _bin_thresholds():
    """C_b = largest f32 y such that f32(15*y) <= b+1, for b = 0..14."""
    thr = []
    for b in range(15):
        tgt = np.float32(b + 1)

        def f(v):
            return np.float32(np.float32(15.0) * v)

        y = np.float32((b + 1) / 15.0)
        if f(y) <= tgt:
            while True:
                y2 = np.nextafter(y, np.float32(np.inf))
                if f(y2) <= tgt:
                    y = y2
                else:
                    break
        else:
            while f(y) > tgt:
                y = np.nextafter(y, np.float32(-np.inf))
        thr.append(np.float32(y))
    thr.append(np.float32(1e9))  # catch-all last segment
    return thr


THR = _bin_thresholds()


def _to_bf16(x):
    """Round f32 -> nearest-even bf16, returned as exactly-representable f32."""
    u = int(np.float32(x).view(np.uint32))
    upper, lower = u >> 16, u & 0xFFFF
    if lower > 0x8000 or (lower == 0x8000 and (upper & 1)):
        upper += 1
    return np.uint32(upper << 16).view(np.float32)


# bf16-representable bin boundaries: the device bins the bf16-rounded conf
# against these, and the host S_b identity uses the same exact values
THRB = [_to_bf16(t) for t in THR[:15]] + [np.float32(2.0 ** 30)]
PAD_CONF_BF = float(_to_bf16(np.float32(np.float32(1.0) * np.float32(1.0 / np.float32(100.0)))))


def _next_bf16_up(x):
    u = int(np.float32(x).view(np.uint32))
    return np.uint32(((u >> 16) + 1) << 16).view(np.float32)


# off-grid thresholds strictly between consecutive bf16 values: a bf16 conf
# can never equal one, so sign(conf - c') is exactly +-1 and
# (N - sum sign)/2 counts {conf <= c_b} exactly
THRP = [
    np.float32(
        np.float64(THRB[b]) + (np.float64(_next_bf16_up(THRB[b])) - np.float64(THRB[b])) / 2
    )
    for b in range(15)
] + [np.float32(1.5 * 2.0 ** 30)]
# u = conf - 2*acc thresholds: {u <= c'-2} == {acc=1 and conf <= c_b}
THRU = [np.float32(np.float32(t) - np.float32(2.0)) for t in THRP[:15]] + [
    np.float32(-0.5)
]


def _build():
    nc = bass.Bass(trn_type="TRN2")
    x = nc.dram_tensor("x", [P, SPP * C], F32, kind="ExternalInput")
    g = nc.dram_tensor("g", [P, SPP], F32, kind="ExternalInput")
    cn = nc.dram_tensor("cn", [P, 32], F32, kind="ExternalInput")
    bins = nc.dram_tensor("bins", [P, 49 * len(CHUNKS)], F32, kind="ExternalOutput")

    with tile.TileContext(nc) as tc:
        BF16 = mybir.dt.bfloat16
        with (
            tc.tile_pool(name="xin", bufs=5) as xin,
            tc.tile_pool(name="persist", bufs=1) as persist,
        ):
            g_sb = persist.tile([P, SPP], F32)
            em = persist.tile([P, SPP], F32)
            s_t = persist.tile([P, SPP], F32)
            rs = persist.tile([P, SPP], F32)
            conf_bf = persist.tile([P, SPP], BF16)
            acc_bf = persist.tile([P, SPP], BF16)
            junk_bf = persist.tile([P, SPP], BF16)
            bins_sb = persist.tile([P, 49 * len(CHUNKS)], F32)
            cn_sb = persist.tile([P, 32], F32)
            nc.gpsimd.dma_start(out=g_sb[:, :], in_=g[:, :])
            nc.gpsimd.dma_start(out=cn_sb[:, :], in_=cn[:, :])

            def emit_chunk(ci, lo, hi):
                sl = slice(lo, hi)
                base = 49 * ci
                nc.vector.reciprocal(rs[:, sl], s_t[:, sl])
                # conf_bf = bf16(em / s): all three per-bin sums see the SAME
                # rounded values, so per-sample bin assignment is consistent;
                # rounding only nudges boundary samples between adjacent bins
                nc.vector.tensor_mul(conf_bf[:, sl], em[:, sl], rs[:, sl])
                nc.scalar.activation(g_sb[:, sl], g_sb[:, sl], ACTF.Exp)
                nc.vector.tensor_tensor(
                    acc_bf[:, sl], g_sb[:, sl], em[:, sl], op=ALU.is_equal
                )
                if ci < N_ACT_CHUNKS:
                    # u = conf - 2*acc  (reuses rs; exact in fp32)
                    nc.vector.scalar_tensor_tensor(
                        rs[:, sl], acc_bf[:, sl], -2.0, conf_bf[:, sl],
                        op0=ALU.mult, op1=ALU.add,
                    )
                junk = s_t  # dead after reciprocal
                if ci < N_ACT_CHUNKS:
                    nc.scalar.activation(
                        junk[:, sl], conf_bf[:, sl], ACTF.Copy,
                        accum_out=bins_sb[:, base + 48 : base + 49],
                    )
                    for b in range(NBINS):
                        nc.scalar.activation(
                            junk[:, sl], conf_bf[:, sl], ACTF.Relu,
                            bias=cn_sb[:, b : b + 1],
                            accum_out=bins_sb[:, base + b : base + b + 1],
                        )
                    for b in range(NBINS):
                        nc.scalar.activation(
                            junk[:, sl], conf_bf[:, sl], ACTF.Sign,
                            bias=cn_sb[:, b : b + 1],
                            accum_out=bins_sb[:, base + 16 + b : base + 17 + b],
                        )
                    for b in range(NBINS):
                        nc.scalar.activation(
                            junk[:, sl], rs[:, sl], ACTF.Sign,
                            bias=cn_sb[:, 16 + b : 17 + b],
                            accum_out=bins_sb[:, base + 32 + b : base + 33 + b],
                        )
                else:
                    # tail chunk: DVE min/count binning (DVE is idle here)
                    for b in range(NBINS):
                        cb = float(THRB[b])
                        nc.vector.tensor_scalar(
                            junk_bf[:, sl], conf_bf[:, sl], cb, None,
                            op0=ALU.min, op1=ALU.add,
                            accum_out=bins_sb[:, base + b : base + b + 1],
                        )
                        nc.vector.tensor_scalar(
                            junk_bf[:, sl], conf_bf[:, sl], cb, None,
                            op0=ALU.is_le, op1=ALU.add,
                            accum_out=bins_sb[:, base + 16 + b : base + 17 + b],
                        )
                        nc.vector.scalar_tensor_tensor(
                            junk_bf[:, sl], conf_bf[:, sl], cb, acc_bf[:, sl],
                            op0=ALU.is_le, op1=ALU.mult,
                            accum_out=bins_sb[:, base + 32 + b : base + 33 + b],
                        )

            dma_engines = [nc.sync, nc.scalar]
            off = 0
            ci = 0
            for t, k in enumerate(SIZES):
                kc = k * C
                xt = xin.tile([P, K * C], F32, tag="xt")
                dma_engines[t % 2].dma_start(
                    out=xt[:, :kc], in_=x[:, off * C : (off + k) * C]
                )
                nc.scalar.activation(xt[:, :kc], xt[:, :kc], ACTF.Exp)
                ev = xt[:, :kc].rearrange("p (k c) -> p k c", c=C)
                nc.vector.reduce_max(
                    out=em[:, off : off + k], in_=ev[:, :, :], axis=AX.X
                )
                nc.vector.reduce_sum(
                    out=s_t[:, off : off + k], in_=ev[:, :, :], axis=AX.X
                )
                off += k
                if ci < len(CHUNKS) and CHUNKS[ci][1] == off:
                    emit_chunk(ci, CHUNKS[ci][0], CHUNKS[ci][1])
                    ci += 1
            assert ci == len(CHUNKS)
            nc.sync.dma_start(out=bins[:, :], in_=bins_sb[:, :])

    # ---- sync-command budget fixes (instructions carry <= 2 sync commands:
    # completion update + at most one wait).  Drop waits provably covered by
    # earlier waits on the same engine, then split any remaining multi-wait
    # instruction across preceding same-engine drains.
    import re as _re

    def _tick_sem(name):
        return bool(_re.match(
            r"^(Activation|DVE|PE|Pool|SP|DMAHW\d+|DMASW\d+)_\d+$", name
        ))

    seen_waits = {}
    for bb in nc.m.functions[0].blocks:
        for ins in bb.instructions:
            si = ins.sync_info
            if si is None:
                continue
            tname = type(ins).__name__
            if tname == "InstEventSemaphore":
                continue
            eng = str(ins.engine).split(".")[-1]
            kept = list(si.on_wait)
            if tname not in ("InstDMACopy", "InstDrain") and len(kept) > 1:
                # same-engine waits are redundant (program order)
                kept = [w for w in kept if not w.ant_name.startswith(f"{eng}_")]
            kept2 = []
            for w in kept:
                if not _tick_sem(w.ant_name):
                    kept2.append(w)
                elif seen_waits.get((eng, w.ant_name), -1) < w.wait_value:
                    kept2.append(w)
            kept = kept2
            for w in kept:
                if not _tick_sem(w.ant_name):
                    continue
                key = (eng, w.ant_name)
                seen_waits[key] = max(seen_waits.get(key, -1), w.wait_value)
            if len(kept) != len(si.on_wait):
                si.on_wait = kept
                ins.sync_info = si

    import bass_rust as _br

    for bb in nc.m.functions[0].blocks:
        while True:
            insns = list(bb.instructions)
            target = None
            for idx, ins in enumerate(insns):
                si = ins.sync_info
                if si is None:
                    continue
                if len(si.on_wait) > 1:
                    target = (idx, ins)
                    break
            if target is None:
                break
            idx, ins = target
            si = ins.sync_info
            waits = list(si.on_wait)
            if type(ins).__name__ == "InstDrain":
                room = max(0, 1 - len(si.on_update))
            else:
                room = 1
            keep, extra = waits[len(waits) - room :], waits[: len(waits) - room]
            pos = idx
            for i, w in enumerate(extra):
                nd = mybir.InstDrain(
                    name=f"{ins.name}-presync{i}", ins=[], outs=[],
                    bass_is_fusable=False,
                )
                nd.engine = ins.engine
                nd.sync_info = _br.SyncInfo(on_wait=[w], on_update=[])
                nc.register_instruction(nd, overwrite=True)
                bb.instructions.insert(pos, nd)
                pos += 1
            si.on_wait = keep
            ins.sync_info = si
    return nc


_NC_CACHE = {}


def _get_nc():
    if "nc" not in _NC_CACHE:
        _NC_CACHE["nc"] = _build()
    return _NC_CACHE["nc"]


def kernel(logits, labels):
    global LAST_RESULTS
    logits = np.ascontiguousarray(np.asarray(logits), dtype=np.float32)
    labels_i = np.asarray(labels).astype(np.int64)
    assert logits.shape == (N, C), logits.shape

    # host-side gather of the label logit (1% of input bytes)
    gvals = logits[np.arange(N), labels_i].astype(np.float32)

    # bias constants: -c'_b for the conf relu/sign passes, then -(c'_b - 2)
    cnrow = np.array(
        [-np.float32(t) for t in THRP] + [-np.float32(t) for t in THRU],
        dtype=np.float32,
    )
    cnarr = np.ascontiguousarray(np.broadcast_to(cnrow, (P, 32)))

    in_maps = []
    for c in range(NCORES):
        lo, hi = c * ROWS, (c + 1) * ROWS
        if hi <= N:
            xs = logits[lo:hi]
            gc = gvals[lo:hi]
        else:
            xs = np.concatenate(
                [logits[lo:], np.zeros((hi - N, C), np.float32)], axis=0
            )
            gc = np.concatenate([gvals[lo:], np.zeros(hi - N, np.float32)])
        in_maps.append(
            {
                "x": np.ascontiguousarray(xs.reshape(P, SPP * C)),
                "g": np.ascontiguousarray(gc.reshape(P, SPP)),
                "cn": cnarr,
            }
        )

    trace = bool(int(os.environ.get("ECE_TRACE", "0")))
    res = run_bass_kernel_spmd(
        _get_nc(), in_maps, core_ids=list(range(NCORES)), trace=trace
    )
    LAST_RESULTS = res

    R = np.zeros(NBINS, np.float64)       # sum relu(conf - c'_b)   [ACT chunks]
    sgc = np.zeros(NBINS, np.float64)     # sum sign(conf - c'_b)
    sgu = np.zeros(NBINS, np.float64)     # sum sign(u - (c'_b - 2))
    sumconf = 0.0
    wt = np.zeros(NBINS, np.float64)      # sum min(conf, c_b)      [DVE chunks]
    nn_d = np.zeros(NBINS, np.float64)    # {conf <= c_b}
    av_d = np.zeros(NBINS, np.float64)    # {acc=1 and conf <= c_b}
    for out in res.results:
        ob = out["bins"].astype(np.float64)
        for ci in range(len(CHUNKS)):
            base = 49 * ci
            if ci < N_ACT_CHUNKS:
                R += ob[:, base : base + 16].sum(axis=0)
                sgc += ob[:, base + 16 : base + 32].sum(axis=0)
                sgu += ob[:, base + 32 : base + 48].sum(axis=0)
                sumconf += ob[:, base + 48].sum()
            else:
                wt += ob[:, base : base + 16].sum(axis=0)
                nn_d += ob[:, base + 16 : base + 32].sum(axis=0)
                av_d += ob[:, base + 32 : base + 48].sum(axis=0)

    # per-scheme sample totals (positional; pads included)
    L_act = sum(hi - lo for ci, (lo, hi) in enumerate(CHUNKS) if ci < N_ACT_CHUNKS)
    L_dve = sum(hi - lo for ci, (lo, hi) in enumerate(CHUNKS) if ci >= N_ACT_CHUNKS)
    L_act *= P * NCORES
    L_dve *= P * NCORES

    nn_a = (L_act - sgc) / 2.0            # {conf <= c_b}, exact counts
    A_a = (L_act - sgu) / 2.0             # {acc=1 and conf <= c_b}
    thrp64 = np.array([np.float64(t) for t in THRP])
    S_a = sumconf - R - thrp64 * (L_act - nn_a)

    thrb64 = np.array([np.float64(t) for t in THRB])
    S_d = wt - thrb64 * (L_dve - nn_d)

    # cumulative totals; then remove the PAD rows (conf_bf = bf16(0.01) <=
    # every threshold, acc=1)
    S = S_a + S_d - PAD * PAD_CONF_BF
    A = A_a + av_d - PAD
    conf_sum = np.diff(S, prepend=0.0)
    acc_sum = np.diff(A, prepend=0.0)
    ece = np.abs(conf_sum - acc_sum).sum() / N
    return np.array([ece], dtype=np.float32)


# revision 37
# speedup vs baseline: 1.0633x; 1.0633x over previous
"""ECE loss kernel for Trainium2 (Bass/Tile), data-parallel over 8 NeuronCores.

Math (per sample row of logits[N, C]):
  conf = max softmax(x) = exp(max(x)) / sum(exp(x))
  acc  = (argmax(x) == label)
  ece  = sum_b |conf_sum[b] - acc_sum[b]| / N     (15 bins + empty tail)

Device pipeline per core (125184 rows as [128 partitions x 978 samples],
tiles of 32 samples x 100 classes):
  - DMA x tiles on the two HWDGE rings (sync/scalar)
  - ACT: E = exp(x) in place
  - DVE: reduce_max + reduce_sum over E per tile (the irreducible 1x passes;
    ~85% of DVE time)
  - binning runs incrementally in 3 sample-chunks so it overlaps the main
    loop.  The first two chunks bin on the ACT engine via accumulate:
      wt-side: sum relu(conf - c') and sum conf  (min(conf,c) identity)
      counts:  sum sign(conf - c') with c' strictly between two bf16 values,
               so sign is exactly +-1 and (L - sum)/2 is an exact count
      acc:     same sign trick on u = conf - 2*acc
    conf is bf16-rounded once (all passes see the same values, so per-sample
    bin assignment is consistent; boundary nudges cancel to ~1e-6 in ECE).
    The small tail chunk bins on the then-idle DVE (min/is_le + accum).
Host: gathers g = x[i,label_i] (1% of bytes), pads 1472 zero rows, merges the
per-core [128, 147] outputs and finishes the tiny 16-bin reduction.
"""

import os

import numpy as np

import concourse.bass as bass
import concourse.mybir as mybir
import concourse.tile as tile
from concourse.bass_utils import run_bass_kernel_spmd

F32 = mybir.dt.float32
ALU = mybir.AluOpType
AX = mybir.AxisListType
ACTF = mybir.ActivationFunctionType

N = 1_000_000
C = 100
NCORES = 8
P = 128
SPP = 978                   # samples per partition (padded)
ROWS = P * SPP              # 125184 rows per core
NTOT = NCORES * ROWS        # 1001472
PAD = NTOT - N              # 1472 zero pad rows (conf=0.01, acc=1)
K = 32                      # samples per tile
SIZES = [16, 16] + [K] * 29 + [18]   # sum = 978; small leading tiles so the
                                     # first reduces start sooner
CHUNKS = [(0, 512), (512, 896), (896, 978)]  # binning chunks (tile-aligned)
N_ACT_CHUNKS = 2            # first chunks bin on ACT (overlap the main loop);
                            # the small tail chunk bins on the then-idle DVE
NBINS = 16

LAST_RESULTS = None


def _bin_thresholds():
    """C_b = largest f32 y such that f32(15*y) <= b+1, for b = 0..14."""
    thr = []
    for b in range(15):
        tgt = np.float32(b + 1)

        def f(v):
            return np.float32(np.float32(15.0) * v)

        y = np.float32((b + 1) / 15.0)
        if f(y) <= tgt:
            while True:
                y2 = np.nextafter(y, np.float32(np.inf))
                if f(y2) <= tgt:
                    y = y2
                else:
                    break
        else:
            while f(y) > tgt:
                y = np.nextafter(y, np.float32(-np.inf))
        thr.append(np.float32(y))
    thr.append(np.float32(1e9))  # catch-all last segment
    return thr


THR = _bin_thresholds()


def _to_bf16(x):
    """Round f32 -> nearest-even bf16, returned as exactly-representable f32."""
    u = int(np.float32(x).view(np.uint32))
    upper, lower = u >> 16, u & 0xFFFF
    if lower > 0x8000 or (lower == 0x8000 and (upper & 1)):
        upper += 1
    return np.uint32(upper << 16).view(np.float32)


# bf16-representable bin boundaries: the device bins the bf16-rounded conf
# against these, and the host S_b identity uses the same exact values
THRB = [_to_bf16(t) for t in THR[:15]] + [np.float32(2.0 ** 30)]
PAD_CONF_BF = float(_to_bf16(np.float32(np.float32(1.0) * np.float32(1.0 / np.float32(100.0)))))


def _next_bf16_up(x):
    u = int(np.float32(x).view(np.uint32))
    return np.uint32(((u >> 16) + 1) << 16).view(np.float32)


# off-grid thresholds strictly between consecutive bf16 values: a bf16 conf
# can never equal one, so sign(conf - c') is exactly +-1 and
# (N - sum sign)/2 counts {conf <= c_b} exactly
THRP = [
    np.float32(
        np.float64(THRB[b]) + (np.float64(_next_bf16_up(THRB[b])) - np.float64(THRB[b])) / 2
    )
    for b in range(15)
] + [np.float32(1.5 * 2.0 ** 30)]
# u = conf - 2*acc thresholds: {u <= c'-2} == {acc=1 and conf <= c_b}
THRU = [np.float32(np.float32(t) - np.float32(2.0)) for t in THRP[:15]] + [
    np.float32(-0.5)
]


def _build():
    nc = bass.Bass(trn_type="TRN2")
    x = nc.dram_tensor("x", [P, SPP * C], F32, kind="ExternalInput")
    g = nc.dram_tensor("g", [P, SPP], F32, kind="ExternalInput")
    cn = nc.dram_tensor("cn", [P, 32], F32, kind="ExternalInput")
    bins = nc.dram_tensor("bins", [P, 49 * len(CHUNKS)], F32, kind="ExternalOutput")

    with tile.TileContext(nc) as tc:
        BF16 = mybir.dt.bfloat16
        with (
            tc.tile_pool(name="xin", bufs=6) as xin,
            tc.tile_pool(name="persist", bufs=1) as persist,
        ):
            g_sb = persist.tile([P, SPP], F32)
            em = persist.tile([P, SPP], F32)
            s_t = persist.tile([P, SPP], F32)
            rs = persist.tile([P, SPP], F32)
            conf_bf = persist.tile([P, SPP], BF16)
            acc_bf = persist.tile([P, SPP], BF16)
            junk_bf = persist.tile([P, SPP], BF16)
            u_t = persist.tile([P, SPP], F32)       # u = conf - 2*acc
            junk_act = persist.tile([P, SPP], F32)  # ACT binning discard out
            bins_sb = persist.tile([P, 49 * len(CHUNKS)], F32)
            cn_sb = persist.tile([P, 32], F32)
            nc.gpsimd.dma_start(out=g_sb[:, :], in_=g[:, :])
            nc.gpsimd.dma_start(out=cn_sb[:, :], in_=cn[:, :])

            def emit_chunk(ci, lo, hi):
                sl = slice(lo, hi)
                base = 49 * ci
                nc.vector.reciprocal(rs[:, sl], s_t[:, sl])
                # conf_bf = bf16(em / s): all three per-bin sums see the SAME
                # rounded values, so per-sample bin assignment is consistent;
                # rounding only nudges boundary samples between adjacent bins
                nc.vector.tensor_mul(conf_bf[:, sl], em[:, sl], rs[:, sl])
                nc.scalar.activation(g_sb[:, sl], g_sb[:, sl], ACTF.Exp)
                nc.vector.tensor_tensor(
                    acc_bf[:, sl], g_sb[:, sl], em[:, sl], op=ALU.is_equal
                )
                if ci < N_ACT_CHUNKS:
                    # u = conf - 2*acc  (exact in fp32)
                    nc.vector.scalar_tensor_tensor(
                        u_t[:, sl], acc_bf[:, sl], -2.0, conf_bf[:, sl],
                        op0=ALU.mult, op1=ALU.add,
                    )
                junk = junk_act
                if ci < N_ACT_CHUNKS:
                    nc.scalar.activation(
                        junk[:, sl], conf_bf[:, sl], ACTF.Copy,
                        accum_out=bins_sb[:, base + 48 : base + 49],
                    )
                    for b in range(NBINS):
                        nc.scalar.activation(
                            junk[:, sl], conf_bf[:, sl], ACTF.Relu,
                            bias=cn_sb[:, b : b + 1],
                            accum_out=bins_sb[:, base + b : base + b + 1],
                        )
                    for b in range(NBINS):
                        nc.scalar.activation(
                            junk[:, sl], conf_bf[:, sl], ACTF.Sign,
                            bias=cn_sb[:, b : b + 1],
                            accum_out=bins_sb[:, base + 16 + b : base + 17 + b],
                        )
                    for b in range(NBINS):
                        nc.scalar.activation(
                            junk[:, sl], u_t[:, sl], ACTF.Sign,
                            bias=cn_sb[:, 16 + b : 17 + b],
                            accum_out=bins_sb[:, base + 32 + b : base + 33 + b],
                        )
                else:
                    # tail chunk: DVE min/count binning (DVE is idle here)
                    for b in range(NBINS):
                        cb = float(THRB[b])
                        nc.vector.tensor_scalar(
                            junk_bf[:, sl], conf_bf[:, sl], cb, None,
                            op0=ALU.min, op1=ALU.add,
                            accum_out=bins_sb[:, base + b : base + b + 1],
                        )
                        nc.vector.tensor_scalar(
                            junk_bf[:, sl], conf_bf[:, sl], cb, None,
                            op0=ALU.is_le, op1=ALU.add,
                            accum_out=bins_sb[:, base + 16 + b : base + 17 + b],
                        )
                        nc.vector.scalar_tensor_tensor(
                            junk_bf[:, sl], conf_bf[:, sl], cb, acc_bf[:, sl],
                            op0=ALU.is_le, op1=ALU.mult,
                            accum_out=bins_sb[:, base + 32 + b : base + 33 + b],
                        )

            dma_engines = [nc.sync, nc.scalar]
            off = 0
            ci = 0
            for t, k in enumerate(SIZES):
                kc = k * C
                xt = xin.tile([P, K * C], F32, tag="xt")
                dma_engines[t % 2].dma_start(
                    out=xt[:, :kc], in_=x[:, off * C : (off + k) * C]
                )
                nc.scalar.activation(xt[:, :kc], xt[:, :kc], ACTF.Exp)
                ev = xt[:, :kc].rearrange("p (k c) -> p k c", c=C)
                nc.vector.reduce_max(
                    out=em[:, off : off + k], in_=ev[:, :, :], axis=AX.X
                )
                nc.vector.reduce_sum(
                    out=s_t[:, off : off + k], in_=ev[:, :, :], axis=AX.X
                )
                off += k
                if ci < len(CHUNKS) and CHUNKS[ci][1] == off:
                    emit_chunk(ci, CHUNKS[ci][0], CHUNKS[ci][1])
                    ci += 1
            assert ci == len(CHUNKS)
            nc.sync.dma_start(out=bins[:, :], in_=bins_sb[:, :])

    # ---- sync-command budget fixes (instructions carry <= 2 sync commands:
    # completion update + at most one wait).  Drop waits provably covered by
    # earlier waits on the same engine, then split any remaining multi-wait
    # instruction across preceding same-engine drains.
    import re as _re

    def _tick_sem(name):
        return bool(_re.match(
            r"^(Activation|DVE|PE|Pool|SP|DMAHW\d+|DMASW\d+)_\d+$", name
        ))

    seen_waits = {}
    for bb in nc.m.functions[0].blocks:
        for ins in bb.instructions:
            si = ins.sync_info
            if si is None:
                continue
            tname = type(ins).__name__
            if tname == "InstEventSemaphore":
                continue
            eng = str(ins.engine).split(".")[-1]
            kept = list(si.on_wait)
            if tname not in ("InstDMACopy", "InstDrain") and len(kept) > 1:
                # same-engine waits are redundant (program order)
                kept = [w for w in kept if not w.ant_name.startswith(f"{eng}_")]
            kept2 = []
            for w in kept:
                if not _tick_sem(w.ant_name):
                    kept2.append(w)
                elif seen_waits.get((eng, w.ant_name), -1) < w.wait_value:
                    kept2.append(w)
            kept = kept2
            for w in kept:
                if not _tick_sem(w.ant_name):
                    continue
                key = (eng, w.ant_name)
                seen_waits[key] = max(seen_waits.get(key, -1), w.wait_value)
            if len(kept) != len(si.on_wait):
                si.on_wait = kept
                ins.sync_info = si

    import bass_rust as _br

    for bb in nc.m.functions[0].blocks:
        while True:
            insns = list(bb.instructions)
            target = None
            for idx, ins in enumerate(insns):
                si = ins.sync_info
                if si is None:
                    continue
                if len(si.on_wait) > 1:
                    target = (idx, ins)
                    break
            if target is None:
                break
            idx, ins = target
            si = ins.sync_info
            waits = list(si.on_wait)
            if type(ins).__name__ == "InstDrain":
                room = max(0, 1 - len(si.on_update))
            else:
                room = 1
            keep, extra = waits[len(waits) - room :], waits[: len(waits) - room]
            pos = idx
            for i, w in enumerate(extra):
                nd = mybir.InstDrain(
                    name=f"{ins.name}-presync{i}", ins=[], outs=[],
                    bass_is_fusable=False,
                )
                nd.engine = ins.engine
                nd.sync_info = _br.SyncInfo(on_wait=[w], on_update=[])
                nc.register_instruction(nd, overwrite=True)
                bb.instructions.insert(pos, nd)
                pos += 1
            si.on_wait = keep
            ins.sync_info = si
    return nc


_NC_CACHE = {}


def _get_nc():
    if "nc" not in _NC_CACHE:
        _NC_CACHE["nc"] = _build()
    return _NC_CACHE["nc"]


def kernel(logits, labels):
    global LAST_RESULTS
    logits = np.ascontiguousarray(np.asarray(logits), dtype=np.float32)
    labels_i = np.asarray(labels).astype(np.int64)
    assert logits.shape == (N, C), logits.shape

    # host-side gather of the label logit (1% of input bytes)
    gvals = logits[np.arange(N), labels_i].astype(np.float32)

    # bias constants: -c'_b for the conf relu/sign passes, then -(c'_b - 2)
    cnrow = np.array(
        [-np.float32(t) for t in THRP] + [-np.float32(t) for t in THRU],
        dtype=np.float32,
    )
    cnarr = np.ascontiguousarray(np.broadcast_to(cnrow, (P, 32)))

    in_maps = []
    for c in range(NCORES):
        lo, hi = c * ROWS, (c + 1) * ROWS
        if hi <= N:
            xs = logits[lo:hi]
            gc = gvals[lo:hi]
        else:
            xs = np.concatenate(
                [logits[lo:], np.zeros((hi - N, C), np.float32)], axis=0
            )
            gc = np.concatenate([gvals[lo:], np.zeros(hi - N, np.float32)])
        in_maps.append(
            {
                "x": np.ascontiguousarray(xs.reshape(P, SPP * C)),
                "g": np.ascontiguousarray(gc.reshape(P, SPP)),
                "cn": cnarr,
            }
        )

    trace = bool(int(os.environ.get("ECE_TRACE", "0")))
    res = run_bass_kernel_spmd(
        _get_nc(), in_maps, core_ids=list(range(NCORES)), trace=trace
    )
    LAST_RESULTS = res

    R = np.zeros(NBINS, np.float64)       # sum relu(conf - c'_b)   [ACT chunks]
    sgc = np.zeros(NBINS, np.float64)     # sum sign(conf - c'_b)
    sgu = np.zeros(NBINS, np.float64)     # sum sign(u - (c'_b - 2))
    sumconf = 0.0
    wt = np.zeros(NBINS, np.float64)      # sum min(conf, c_b)      [DVE chunks]
    nn_d = np.zeros(NBINS, np.float64)    # {conf <= c_b}
    av_d = np.zeros(NBINS, np.float64)    # {acc=1 and conf <= c_b}
    for out in res.results:
        ob = out["bins"].astype(np.float64)
        for ci in range(len(CHUNKS)):
            base = 49 * ci
            if ci < N_ACT_CHUNKS:
                R += ob[:, base : base + 16].sum(axis=0)
                sgc += ob[:, base + 16 : base + 32].sum(axis=0)
                sgu += ob[:, base + 32 : base + 48].sum(axis=0)
                sumconf += ob[:, base + 48].sum()
            else:
                wt += ob[:, base : base + 16].sum(axis=0)
                nn_d += ob[:, base + 16 : base + 32].sum(axis=0)
                av_d += ob[:, base + 32 : base + 48].sum(axis=0)

    # per-scheme sample totals (positional; pads included)
    L_act = sum(hi - lo for ci, (lo, hi) in enumerate(CHUNKS) if ci < N_ACT_CHUNKS)
    L_dve = sum(hi - lo for ci, (lo, hi) in enumerate(CHUNKS) if ci >= N_ACT_CHUNKS)
    L_act *= P * NCORES
    L_dve *= P * NCORES

    nn_a = (L_act - sgc) / 2.0            # {conf <= c_b}, exact counts
    A_a = (L_act - sgu) / 2.0             # {acc=1 and conf <= c_b}
    thrp64 = np.array([np.float64(t) for t in THRP])
    S_a = sumconf - R - thrp64 * (L_act - nn_a)

    thrb64 = np.array([np.float64(t) for t in THRB])
    S_d = wt - thrb64 * (L_dve - nn_d)

    # cumulative totals; then remove the PAD rows (conf_bf = bf16(0.01) <=
    # every threshold, acc=1)
    S = S_a + S_d - PAD * PAD_CONF_BF
    A = A_a + av_d - PAD
    conf_sum = np.diff(S, prepend=0.0)
    acc_sum = np.diff(A, prepend=0.0)
    ece = np.abs(conf_sum - acc_sum).sum() / N
    return np.array([ece], dtype=np.float32)


# revision 42
# speedup vs baseline: 1.0691x; 1.0054x over previous
"""ECE loss kernel for Trainium2 (Bass/Tile), data-parallel over 8 NeuronCores.

Math (per sample row of logits[N, C]):
  conf = max softmax(x) = exp(max(x)) / sum(exp(x))
  acc  = (argmax(x) == label)
  ece  = sum_b |conf_sum[b] - acc_sum[b]| / N     (15 bins + empty tail)

Device pipeline per core (125184 rows as [128 partitions x 978 samples],
tiles of 32 samples x 100 classes):
  - DMA x tiles on the two HWDGE rings (sync/scalar)
  - ACT: E = exp(x) in place
  - DVE: reduce_max + reduce_sum over E per tile (the irreducible 1x passes;
    ~85% of DVE time)
  - binning runs incrementally in 3 sample-chunks so it overlaps the main
    loop.  The first two chunks bin on the ACT engine via accumulate:
      wt-side: sum relu(conf - c') and sum conf  (min(conf,c) identity)
      counts:  sum sign(conf - c') with c' strictly between two bf16 values,
               so sign is exactly +-1 and (L - sum)/2 is an exact count
      acc:     same sign trick on u = conf - 2*acc
    conf is bf16-rounded once (all passes see the same values, so per-sample
    bin assignment is consistent; boundary nudges cancel to ~1e-6 in ECE).
    The small tail chunk bins on the then-idle DVE (min/is_le + accum).
Host: gathers g = x[i,label_i] (1% of bytes), pads 1472 zero rows, merges the
per-core [128, 147] outputs and finishes the tiny 16-bin reduction.
"""

import os

import numpy as np

import concourse.bass as bass
import concourse.mybir as mybir
import concourse.tile as tile
from concourse.bass_utils import run_bass_kernel_spmd

F32 = mybir.dt.float32
ALU = mybir.AluOpType
AX = mybir.AxisListType
ACTF = mybir.ActivationFunctionType

N = 1_000_000
C = 100
NCORES = 8
P = 128
SPP = 978                   # samples per partition (padded)
ROWS = P * SPP              # 125184 rows per core
NTOT = NCORES * ROWS        # 1001472
PAD = NTOT - N              # 1472 zero pad rows (conf=0.01, acc=1)
K = 32                      # samples per tile
SIZES = [16, 16] + [K] * 29 + [18]   # sum = 978; small leading tiles so the
                                     # first reduces start sooner
CHUNKS = [(0, 512), (512, 896), (896, 978)]  # binning chunks (tile-aligned)
N_ACT_CHUNKS = 2            # first chunks bin on ACT (overlap the main loop);
                            # the small tail chunk bins on the then-idle DVE
NBINS = 16

LAST_RESULTS = None


def _bin_thresholds():
    """C_b = largest f32 y such that f32(15*y) <= b+1, for b = 0..14."""
    thr = []
    for b in range(15):
        tgt = np.float32(b + 1)

        def f(v):
            return np.float32(np.float32(15.0) * v)

        y = np.float32((b + 1) / 15.0)
        if f(y) <= tgt:
            while True:
                y2 = np.nextafter(y, np.float32(np.inf))
                if f(y2) <= tgt:
                    y = y2
                else:
                    break
        else:
            while f(y) > tgt:
                y = np.nextafter(y, np.float32(-np.inf))
        thr.append(np.float32(y))
    thr.append(np.float32(1e9))  # catch-all last segment
    return thr


THR = _bin_thresholds()


def _to_bf16(x):
    """Round f32 -> nearest-even bf16, returned as exactly-representable f32."""
    u = int(np.float32(x).view(np.uint32))
    upper, lower = u >> 16, u & 0xFFFF
    if lower > 0x8000 or (lower == 0x8000 and (upper & 1)):
        upper += 1
    return np.uint32(upper << 16).view(np.float32)


# bf16-representable bin boundaries: the device bins the bf16-rounded conf
# against these, and the host S_b identity uses the same exact values
THRB = [_to_bf16(t) for t in THR[:15]] + [np.float32(2.0 ** 30)]
PAD_CONF_BF = float(_to_bf16(np.float32(np.float32(1.0) * np.float32(1.0 / np.float32(100.0)))))


def _next_bf16_up(x):
    u = int(np.float32(x).view(np.uint32))
    return np.uint32(((u >> 16) + 1) << 16).view(np.float32)


# off-grid thresholds strictly between consecutive bf16 values: a bf16 conf
# can never equal one, so sign(conf - c') is exactly +-1 and
# (N - sum sign)/2 counts {conf <= c_b} exactly
THRP = [
    np.float32(
        np.float64(THRB[b]) + (np.float64(_next_bf16_up(THRB[b])) - np.float64(THRB[b])) / 2
    )
    for b in range(15)
] + [np.float32(1.5 * 2.0 ** 30)]
# u = conf - 2*acc thresholds: {u <= c'-2} == {acc=1 and conf <= c_b}
THRU = [np.float32(np.float32(t) - np.float32(2.0)) for t in THRP[:15]] + [
    np.float32(-0.5)
]


def _build():
    nc = bass.Bass(trn_type="TRN2")
    x = nc.dram_tensor("x", [P, SPP * C], F32, kind="ExternalInput")
    g = nc.dram_tensor("g", [P, SPP], F32, kind="ExternalInput")
    cn = nc.dram_tensor("cn", [P, 32], F32, kind="ExternalInput")
    bins = nc.dram_tensor("bins", [P, 49 * len(CHUNKS)], F32, kind="ExternalOutput")

    with tile.TileContext(nc) as tc:
        BF16 = mybir.dt.bfloat16
        with (
            tc.tile_pool(name="xin", bufs=6) as xin,
            tc.tile_pool(name="persist", bufs=1) as persist,
        ):
            g_sb = persist.tile([P, SPP], F32)
            em = persist.tile([P, SPP], F32)
            s_t = persist.tile([P, SPP], F32)
            rs = persist.tile([P, SPP], F32)
            conf_bf = persist.tile([P, SPP], BF16)
            acc_bf = persist.tile([P, SPP], BF16)
            junk_bf = persist.tile([P, SPP], BF16)
            u_t = persist.tile([P, SPP], F32)       # u = conf - 2*acc
            junk_act = persist.tile([P, SPP], F32)  # ACT binning discard out
            # separate accumulator tiles per engine so the DVE tail binning
            # never serializes behind the ACT chunks' accumulate writes
            bins_a = persist.tile([P, 49 * N_ACT_CHUNKS], F32)
            bins_d = persist.tile([P, 49 * (len(CHUNKS) - N_ACT_CHUNKS)], F32)
            cn_sb = persist.tile([P, 32], F32)
            nc.gpsimd.dma_start(out=g_sb[:, :], in_=g[:, :])
            nc.gpsimd.dma_start(out=cn_sb[:, :], in_=cn[:, :])
            # exp(g) depends on nothing tile-wise: run it once up front so no
            # chunk's accuracy compare ever waits behind ACT's binning bursts
            nc.scalar.activation(g_sb[:, :], g_sb[:, :], ACTF.Exp)

            def emit_chunk(ci, lo, hi):
                sl = slice(lo, hi)
                if ci < N_ACT_CHUNKS:
                    bins_sb, base = bins_a, 49 * ci
                else:
                    bins_sb, base = bins_d, 49 * (ci - N_ACT_CHUNKS)
                nc.vector.reciprocal(rs[:, sl], s_t[:, sl])
                # conf_bf = bf16(em / s): all three per-bin sums see the SAME
                # rounded values, so per-sample bin assignment is consistent;
                # rounding only nudges boundary samples between adjacent bins
                nc.vector.tensor_mul(conf_bf[:, sl], em[:, sl], rs[:, sl])
                nc.vector.tensor_tensor(
                    acc_bf[:, sl], g_sb[:, sl], em[:, sl], op=ALU.is_equal
                )
                if ci < N_ACT_CHUNKS:
                    # u = conf - 2*acc  (exact in fp32)
                    nc.vector.scalar_tensor_tensor(
                        u_t[:, sl], acc_bf[:, sl], -2.0, conf_bf[:, sl],
                        op0=ALU.mult, op1=ALU.add,
                    )
                junk = junk_act
                if ci < N_ACT_CHUNKS:
                    nc.scalar.activation(
                        junk[:, sl], conf_bf[:, sl], ACTF.Copy,
                        accum_out=bins_sb[:, base + 48 : base + 49],
                    )
                    for b in range(NBINS):
                        nc.scalar.activation(
                            junk[:, sl], conf_bf[:, sl], ACTF.Relu,
                            bias=cn_sb[:, b : b + 1],
                            accum_out=bins_sb[:, base + b : base + b + 1],
                        )
                    for b in range(NBINS):
                        nc.scalar.activation(
                            junk[:, sl], conf_bf[:, sl], ACTF.Sign,
                            bias=cn_sb[:, b : b + 1],
                            accum_out=bins_sb[:, base + 16 + b : base + 17 + b],
                        )
                    for b in range(NBINS):
                        nc.scalar.activation(
                            junk[:, sl], u_t[:, sl], ACTF.Sign,
                            bias=cn_sb[:, 16 + b : 17 + b],
                            accum_out=bins_sb[:, base + 32 + b : base + 33 + b],
                        )
                else:
                    # tail chunk: DVE min/count binning (DVE is idle here)
                    for b in range(NBINS):
                        cb = float(THRB[b])
                        nc.vector.tensor_scalar(
                            junk_bf[:, sl], conf_bf[:, sl], cb, None,
                            op0=ALU.min, op1=ALU.add,
                            accum_out=bins_sb[:, base + b : base + b + 1],
                        )
                        nc.vector.tensor_scalar(
                            junk_bf[:, sl], conf_bf[:, sl], cb, None,
                            op0=ALU.is_le, op1=ALU.add,
                            accum_out=bins_sb[:, base + 16 + b : base + 17 + b],
                        )
                        nc.vector.scalar_tensor_tensor(
                            junk_bf[:, sl], conf_bf[:, sl], cb, acc_bf[:, sl],
                            op0=ALU.is_le, op1=ALU.mult,
                            accum_out=bins_sb[:, base + 32 + b : base + 33 + b],
                        )

            dma_engines = [nc.sync, nc.scalar]
            off = 0
            ci = 0
            for t, k in enumerate(SIZES):
                kc = k * C
                xt = xin.tile([P, K * C], F32, tag="xt")
                dma_engines[t % 2].dma_start(
                    out=xt[:, :kc], in_=x[:, off * C : (off + k) * C]
                )
                nc.scalar.activation(xt[:, :kc], xt[:, :kc], ACTF.Exp)
                ev = xt[:, :kc].rearrange("p (k c) -> p k c", c=C)
                nc.vector.reduce_max(
                    out=em[:, off : off + k], in_=ev[:, :, :], axis=AX.X
                )
                nc.vector.reduce_sum(
                    out=s_t[:, off : off + k], in_=ev[:, :, :], axis=AX.X
                )
                off += k
                if ci < len(CHUNKS) and CHUNKS[ci][1] == off:
                    # high priority: run the chunk's conf/acc ops ahead of
                    # later tiles' reduces so the ACT binning burst starts
                    # as early as its data allows
                    with tc.high_priority():
                        emit_chunk(ci, CHUNKS[ci][0], CHUNKS[ci][1])
                    ci += 1
            assert ci == len(CHUNKS)
            nc.sync.dma_start(out=bins[:, 0 : 49 * N_ACT_CHUNKS], in_=bins_a[:, :])
            nc.sync.dma_start(out=bins[:, 49 * N_ACT_CHUNKS :], in_=bins_d[:, :])

    # ---- sync-command budget fixes (instructions carry <= 2 sync commands:
    # completion update + at most one wait).  Drop waits provably covered by
    # earlier waits on the same engine, then split any remaining multi-wait
    # instruction across preceding same-engine drains.
    import re as _re

    def _tick_sem(name):
        return bool(_re.match(
            r"^(Activation|DVE|PE|Pool|SP|DMAHW\d+|DMASW\d+)_\d+$", name
        ))

    seen_waits = {}
    for bb in nc.m.functions[0].blocks:
        for ins in bb.instructions:
            si = ins.sync_info
            if si is None:
                continue
            tname = type(ins).__name__
            if tname == "InstEventSemaphore":
                continue
            eng = str(ins.engine).split(".")[-1]
            kept = list(si.on_wait)
            if tname not in ("InstDMACopy", "InstDrain") and len(kept) > 1:
                # same-engine waits are redundant (program order)
                kept = [w for w in kept if not w.ant_name.startswith(f"{eng}_")]
            kept2 = []
            for w in kept:
                if not _tick_sem(w.ant_name):
                    kept2.append(w)
                elif seen_waits.get((eng, w.ant_name), -1) < w.wait_value:
                    kept2.append(w)
            kept = kept2
            for w in kept:
                if not _tick_sem(w.ant_name):
                    continue
                key = (eng, w.ant_name)
                seen_waits[key] = max(seen_waits.get(key, -1), w.wait_value)
            if len(kept) != len(si.on_wait):
                si.on_wait = kept
                ins.sync_info = si

    import bass_rust as _br

    for bb in nc.m.functions[0].blocks:
        while True:
            insns = list(bb.instructions)
            target = None
            for idx, ins in enumerate(insns):
                si = ins.sync_info
                if si is None:
                    continue
                if len(si.on_wait) > 1:
                    target = (idx, ins)
                    break
            if target is None:
                break
            idx, ins = target
            si = ins.sync_info
            waits = list(si.on_wait)
            if type(ins).__name__ == "InstDrain":
                room = max(0, 1 - len(si.on_update))
            else:
                room = 1
            keep, extra = waits[len(waits) - room :], waits[: len(waits) - room]
            pos = idx
            for i, w in enumerate(extra):
                nd = mybir.InstDrain(
                    name=f"{ins.name}-presync{i}", ins=[], outs=[],
                    bass_is_fusable=False,
                )
                nd.engine = ins.engine
                nd.sync_info = _br.SyncInfo(on_wait=[w], on_update=[])
                nc.register_instruction(nd, overwrite=True)
                bb.instructions.insert(pos, nd)
                pos += 1
            si.on_wait = keep
            ins.sync_info = si
    return nc


_NC_CACHE = {}


def _get_nc():
    if "nc" not in _NC_CACHE:
        _NC_CACHE["nc"] = _build()
    return _NC_CACHE["nc"]


def kernel(logits, labels):
    global LAST_RESULTS
    logits = np.ascontiguousarray(np.asarray(logits), dtype=np.float32)
    labels_i = np.asarray(labels).astype(np.int64)
    assert logits.shape == (N, C), logits.shape

    # host-side gather of the label logit (1% of input bytes)
    gvals = logits[np.arange(N), labels_i].astype(np.float32)

    # bias constants: -c'_b for the conf relu/sign passes, then -(c'_b - 2)
    cnrow = np.array(
        [-np.float32(t) for t in THRP] + [-np.float32(t) for t in THRU],
        dtype=np.float32,
    )
    cnarr = np.ascontiguousarray(np.broadcast_to(cnrow, (P, 32)))

    in_maps = []
    for c in range(NCORES):
        lo, hi = c * ROWS, (c + 1) * ROWS
        if hi <= N:
            xs = logits[lo:hi]
            gc = gvals[lo:hi]
        else:
            xs = np.concatenate(
                [logits[lo:], np.zeros((hi - N, C), np.float32)], axis=0
            )
            gc = np.concatenate([gvals[lo:], np.zeros(hi - N, np.float32)])
        in_maps.append(
            {
                "x": np.ascontiguousarray(xs.reshape(P, SPP * C)),
                "g": np.ascontiguousarray(gc.reshape(P, SPP)),
                "cn": cnarr,
            }
        )

    trace = bool(int(os.environ.get("ECE_TRACE", "0")))
    res = run_bass_kernel_spmd(
        _get_nc(), in_maps, core_ids=list(range(NCORES)), trace=trace
    )
    LAST_RESULTS = res

    R = np.zeros(NBINS, np.float64)       # sum relu(conf - c'_b)   [ACT chunks]
    sgc = np.zeros(NBINS, np.float64)     # sum sign(conf - c'_b)
    sgu = np.zeros(NBINS, np.float64)     # sum sign(u - (c'_b - 2))
    sumconf = 0.0
    wt = np.zeros(NBINS, np.float64)      # sum min(conf, c_b)      [DVE chunks]
    nn_d = np.zeros(NBINS, np.float64)    # {conf <= c_b}
    av_d = np.zeros(NBINS, np.float64)    # {acc=1 and conf <= c_b}
    for out in res.results:
        ob = out["bins"].astype(np.float64)
        for ci in range(len(CHUNKS)):
            base = 49 * ci
            if ci < N_ACT_CHUNKS:
                R += ob[:, base : base + 16].sum(axis=0)
                sgc += ob[:, base + 16 : base + 32].sum(axis=0)
                sgu += ob[:, base + 32 : base + 48].sum(axis=0)
                sumconf += ob[:, base + 48].sum()
            else:
                wt += ob[:, base : base + 16].sum(axis=0)
                nn_d += ob[:, base + 16 : base + 32].sum(axis=0)
                av_d += ob[:, base + 32 : base + 48].sum(axis=0)

    # per-scheme sample totals (positional; pads included)
    L_act = sum(hi - lo for ci, (lo, hi) in enumerate(CHUNKS) if ci < N_ACT_CHUNKS)
    L_dve = sum(hi - lo for ci, (lo, hi) in enumerate(CHUNKS) if ci >= N_ACT_CHUNKS)
    L_act *= P * NCORES
    L_dve *= P * NCORES

    nn_a = (L_act - sgc) / 2.0            # {conf <= c_b}, exact counts
    A_a = (L_act - sgu) / 2.0             # {acc=1 and conf <= c_b}
    thrp64 = np.array([np.float64(t) for t in THRP])
    S_a = sumconf - R - thrp64 * (L_act - nn_a)

    thrb64 = np.array([np.float64(t) for t in THRB])
    S_d = wt - thrb64 * (L_dve - nn_d)

    # cumulative totals; then remove the PAD rows (conf_bf = bf16(0.01) <=
    # every threshold, acc=1)
    S = S_a + S_d - PAD * PAD_CONF_BF
    A = A_a + av_d - PAD
    conf_sum = np.diff(S, prepend=0.0)
    acc_sum = np.diff(A, prepend=0.0)
    ece = np.abs(conf_sum - acc_sum).sum() / N
    return np.array([ece], dtype=np.float32)


# revision 46
# speedup vs baseline: 1.1010x; 1.0299x over previous
"""ECE loss kernel for Trainium2 (Bass/Tile), data-parallel over 8 NeuronCores.

Math (per sample row of logits[N, C]):
  conf = max softmax(x) = exp(max(x)) / sum(exp(x))
  acc  = (argmax(x) == label)
  ece  = sum_b |conf_sum[b] - acc_sum[b]| / N     (15 bins + empty tail)

Device pipeline per core (125184 rows as [128 partitions x 978 samples],
tiles of 32 samples x 100 classes):
  - DMA x tiles on the two HWDGE rings (sync/scalar)
  - ACT: E = exp(x) in place
  - DVE: reduce_max + reduce_sum over E per tile (the irreducible 1x passes;
    ~85% of DVE time)
  - binning runs incrementally in 3 sample-chunks so it overlaps the main
    loop.  The first two chunks bin on the ACT engine via accumulate:
      wt-side: sum relu(conf - c') and sum conf  (min(conf,c) identity)
      counts:  sum sign(conf - c') with c' strictly between two bf16 values,
               so sign is exactly +-1 and (L - sum)/2 is an exact count
      acc:     same sign trick on u = conf - 2*acc
    conf is bf16-rounded once (all passes see the same values, so per-sample
    bin assignment is consistent; boundary nudges cancel to ~1e-6 in ECE).
    The small tail chunk bins on the then-idle DVE (min/is_le + accum).
Host: gathers g = x[i,label_i] (1% of bytes), pads 1472 zero rows, merges the
per-core [128, 147] outputs and finishes the tiny 16-bin reduction.
"""

import os

import numpy as np

import concourse.bass as bass
import concourse.mybir as mybir
import concourse.tile as tile
from concourse.bass_utils import run_bass_kernel_spmd

F32 = mybir.dt.float32
ALU = mybir.AluOpType
AX = mybir.AxisListType
ACTF = mybir.ActivationFunctionType

N = 1_000_000
C = 100
NCORES = 8
P = 128
SPP = 978                   # samples per partition (padded)
ROWS = P * SPP              # 125184 rows per core
NTOT = NCORES * ROWS        # 1001472
PAD = NTOT - N              # 1472 zero pad rows (conf=0.01, acc=1)
K = 32                      # samples per tile
SIZES = [16, 16] + [K] * 29 + [18]   # sum = 978; small leading tiles so the
                                     # first reduces start sooner
CHUNKS = [(0, 512), (512, 896), (896, 978)]  # binning chunks (tile-aligned)
N_ACT_CHUNKS = 2            # first chunks bin on ACT (overlap the main loop);
                            # the small tail chunk bins on the then-idle DVE
NBINS = 16

LAST_RESULTS = None


def _bin_thresholds():
    """C_b = largest f32 y such that f32(15*y) <= b+1, for b = 0..14."""
    thr = []
    for b in range(15):
        tgt = np.float32(b + 1)

        def f(v):
            return np.float32(np.float32(15.0) * v)

        y = np.float32((b + 1) / 15.0)
        if f(y) <= tgt:
            while True:
                y2 = np.nextafter(y, np.float32(np.inf))
                if f(y2) <= tgt:
                    y = y2
                else:
                    break
        else:
            while f(y) > tgt:
                y = np.nextafter(y, np.float32(-np.inf))
        thr.append(np.float32(y))
    thr.append(np.float32(1e9))  # catch-all last segment
    return thr


THR = _bin_thresholds()


def _to_bf16(x):
    """Round f32 -> nearest-even bf16, returned as exactly-representable f32."""
    u = int(np.float32(x).view(np.uint32))
    upper, lower = u >> 16, u & 0xFFFF
    if lower > 0x8000 or (lower == 0x8000 and (upper & 1)):
        upper += 1
    return np.uint32(upper << 16).view(np.float32)


# bf16-representable bin boundaries: the device bins the bf16-rounded conf
# against these, and the host S_b identity uses the same exact values
THRB = [_to_bf16(t) for t in THR[:15]] + [np.float32(2.0 ** 30)]
PAD_CONF_BF = float(_to_bf16(np.float32(np.float32(1.0) * np.float32(1.0 / np.float32(100.0)))))


def _next_bf16_up(x):
    u = int(np.float32(x).view(np.uint32))
    return np.uint32(((u >> 16) + 1) << 16).view(np.float32)


# off-grid thresholds strictly between consecutive bf16 values: a bf16 conf
# can never equal one, so sign(conf - c') is exactly +-1 and
# (N - sum sign)/2 counts {conf <= c_b} exactly
THRP = [
    np.float32(
        np.float64(THRB[b]) + (np.float64(_next_bf16_up(THRB[b])) - np.float64(THRB[b])) / 2
    )
    for b in range(15)
] + [np.float32(1.5 * 2.0 ** 30)]
# u = conf - 2*acc thresholds: {u <= c'-2} == {acc=1 and conf <= c_b}
THRU = [np.float32(np.float32(t) - np.float32(2.0)) for t in THRP[:15]] + [
    np.float32(-0.5)
]


def _build():
    nc = bass.Bass(trn_type="TRN2")
    x = nc.dram_tensor("x", [P, SPP * C], F32, kind="ExternalInput")
    g = nc.dram_tensor("g", [P, SPP], F32, kind="ExternalInput")
    cn = nc.dram_tensor("cn", [P, 32], F32, kind="ExternalInput")
    bins = nc.dram_tensor(
        "bins", [P, 49 * len(CHUNKS) + 16], F32, kind="ExternalOutput"
    )

    with tile.TileContext(nc) as tc:
        BF16 = mybir.dt.bfloat16
        with (
            tc.tile_pool(name="xin", bufs=6) as xin,
            tc.tile_pool(name="persist", bufs=1) as persist,
        ):
            g_sb = persist.tile([P, SPP], F32)
            em = persist.tile([P, SPP], F32)
            s_t = persist.tile([P, SPP], F32)
            rs = persist.tile([P, SPP], F32)
            conf_bf = persist.tile([P, SPP], BF16)
            acc_bf = persist.tile([P, SPP], BF16)
            junk_bf = persist.tile([P, SPP], BF16)
            u_t = persist.tile([P, SPP], F32)       # u = conf - 2*acc
            junk_act = persist.tile([P, SPP], F32)  # ACT binning discard out
            # separate accumulator tiles per engine so the DVE tail binning
            # never serializes behind the ACT chunks' accumulate writes
            bins_a = persist.tile([P, 49 * N_ACT_CHUNKS], F32)
            # 49 cols for the DVE tail chunk + 16 for chunk-1's wt sums
            # (its relu ops run on DVE so ACT's final burst is sign-only)
            bins_d = persist.tile([P, 49 * (len(CHUNKS) - N_ACT_CHUNKS) + 16], F32)
            cn_sb = persist.tile([P, 32], F32)
            nc.gpsimd.dma_start(out=g_sb[:, :], in_=g[:, :])
            nc.gpsimd.dma_start(out=cn_sb[:, :], in_=cn[:, :])
            # exp(g) depends on nothing tile-wise: run it once up front so no
            # chunk's accuracy compare ever waits behind ACT's binning bursts
            nc.scalar.activation(g_sb[:, :], g_sb[:, :], ACTF.Exp)

            def emit_chunk(ci, lo, hi):
                sl = slice(lo, hi)
                if ci < N_ACT_CHUNKS:
                    bins_sb, base = bins_a, 49 * ci
                else:
                    bins_sb, base = bins_d, 49 * (ci - N_ACT_CHUNKS)
                nc.vector.reciprocal(rs[:, sl], s_t[:, sl])
                # conf_bf = bf16(em / s): all three per-bin sums see the SAME
                # rounded values, so per-sample bin assignment is consistent;
                # rounding only nudges boundary samples between adjacent bins
                nc.vector.tensor_mul(conf_bf[:, sl], em[:, sl], rs[:, sl])
                nc.vector.tensor_tensor(
                    acc_bf[:, sl], g_sb[:, sl], em[:, sl], op=ALU.is_equal
                )
                if ci < N_ACT_CHUNKS:
                    # u = conf - 2*acc  (exact in fp32)
                    nc.vector.scalar_tensor_tensor(
                        u_t[:, sl], acc_bf[:, sl], -2.0, conf_bf[:, sl],
                        op0=ALU.mult, op1=ALU.add,
                    )
                junk = junk_act
                if ci < N_ACT_CHUNKS:
                    if ci < N_ACT_CHUNKS - 1:
                        nc.scalar.activation(
                            junk[:, sl], conf_bf[:, sl], ACTF.Copy,
                            accum_out=bins_sb[:, base + 48 : base + 49],
                        )
                        for b in range(NBINS):
                            nc.scalar.activation(
                                junk[:, sl], conf_bf[:, sl], ACTF.Relu,
                                bias=cn_sb[:, b : b + 1],
                                accum_out=bins_sb[:, base + b : base + b + 1],
                            )
                    else:
                        # last ACT chunk: wt via DVE min-trick in the tail
                        # window, so ACT's closing burst is sign-ops only
                        for b in range(NBINS):
                            nc.vector.tensor_scalar(
                                junk_bf[:, sl], conf_bf[:, sl],
                                float(THRB[b]), None,
                                op0=ALU.min, op1=ALU.add,
                                accum_out=bins_d[:, 49 + b : 50 + b],
                            )
                    for b in range(NBINS):
                        nc.scalar.activation(
                            junk[:, sl], conf_bf[:, sl], ACTF.Sign,
                            bias=cn_sb[:, b : b + 1],
                            accum_out=bins_sb[:, base + 16 + b : base + 17 + b],
                        )
                    for b in range(NBINS):
                        nc.scalar.activation(
                            junk[:, sl], u_t[:, sl], ACTF.Sign,
                            bias=cn_sb[:, 16 + b : 17 + b],
                            accum_out=bins_sb[:, base + 32 + b : base + 33 + b],
                        )
                else:
                    # tail chunk: DVE min/count binning (DVE is idle here)
                    for b in range(NBINS):
                        cb = float(THRB[b])
                        nc.vector.tensor_scalar(
                            junk_bf[:, sl], conf_bf[:, sl], cb, None,
                            op0=ALU.min, op1=ALU.add,
                            accum_out=bins_sb[:, base + b : base + b + 1],
                        )
                        nc.vector.tensor_scalar(
                            junk_bf[:, sl], conf_bf[:, sl], cb, None,
                            op0=ALU.is_le, op1=ALU.add,
                            accum_out=bins_sb[:, base + 16 + b : base + 17 + b],
                        )
                        nc.vector.scalar_tensor_tensor(
                            junk_bf[:, sl], conf_bf[:, sl], cb, acc_bf[:, sl],
                            op0=ALU.is_le, op1=ALU.mult,
                            accum_out=bins_sb[:, base + 32 + b : base + 33 + b],
                        )

            dma_engines = [nc.sync, nc.scalar]
            off = 0
            ci = 0
            for t, k in enumerate(SIZES):
                kc = k * C
                xt = xin.tile([P, K * C], F32, tag="xt")
                dma_engines[t % 2].dma_start(
                    out=xt[:, :kc], in_=x[:, off * C : (off + k) * C]
                )
                nc.scalar.activation(xt[:, :kc], xt[:, :kc], ACTF.Exp)
                ev = xt[:, :kc].rearrange("p (k c) -> p k c", c=C)
                nc.vector.reduce_max(
                    out=em[:, off : off + k], in_=ev[:, :, :], axis=AX.X
                )
                nc.vector.reduce_sum(
                    out=s_t[:, off : off + k], in_=ev[:, :, :], axis=AX.X
                )
                off += k
                if ci < len(CHUNKS) and CHUNKS[ci][1] == off:
                    # high priority: run the chunk's conf/acc ops ahead of
                    # later tiles' reduces so the ACT binning burst starts
                    # as early as its data allows
                    with tc.high_priority():
                        emit_chunk(ci, CHUNKS[ci][0], CHUNKS[ci][1])
                    ci += 1
            assert ci == len(CHUNKS)
            nc.sync.dma_start(out=bins[:, 0 : 49 * N_ACT_CHUNKS], in_=bins_a[:, :])
            nc.sync.dma_start(out=bins[:, 49 * N_ACT_CHUNKS :], in_=bins_d[:, :])

    # ---- sync-command budget fixes (instructions carry <= 2 sync commands:
    # completion update + at most one wait).  Drop waits provably covered by
    # earlier waits on the same engine, then split any remaining multi-wait
    # instruction across preceding same-engine drains.
    import re as _re

    def _tick_sem(name):
        return bool(_re.match(
            r"^(Activation|DVE|PE|Pool|SP|DMAHW\d+|DMASW\d+)_\d+$", name
        ))

    seen_waits = {}
    for bb in nc.m.functions[0].blocks:
        for ins in bb.instructions:
            si = ins.sync_info
            if si is None:
                continue
            tname = type(ins).__name__
            if tname == "InstEventSemaphore":
                continue
            eng = str(ins.engine).split(".")[-1]
            kept = list(si.on_wait)
            if tname not in ("InstDMACopy", "InstDrain") and len(kept) > 1:
                # same-engine waits are redundant (program order)
                kept = [w for w in kept if not w.ant_name.startswith(f"{eng}_")]
            kept2 = []
            for w in kept:
                if not _tick_sem(w.ant_name):
                    kept2.append(w)
                elif seen_waits.get((eng, w.ant_name), -1) < w.wait_value:
                    kept2.append(w)
            kept = kept2
            for w in kept:
                if not _tick_sem(w.ant_name):
                    continue
                key = (eng, w.ant_name)
                seen_waits[key] = max(seen_waits.get(key, -1), w.wait_value)
            if len(kept) != len(si.on_wait):
                si.on_wait = kept
                ins.sync_info = si

    import bass_rust as _br

    for bb in nc.m.functions[0].blocks:
        while True:
            insns = list(bb.instructions)
            target = None
            for idx, ins in enumerate(insns):
                si = ins.sync_info
                if si is None:
                    continue
                if len(si.on_wait) > 1:
                    target = (idx, ins)
                    break
            if target is None:
                break
            idx, ins = target
            si = ins.sync_info
            waits = list(si.on_wait)
            if type(ins).__name__ == "InstDrain":
                room = max(0, 1 - len(si.on_update))
            else:
                room = 1
            keep, extra = waits[len(waits) - room :], waits[: len(waits) - room]
            pos = idx
            for i, w in enumerate(extra):
                nd = mybir.InstDrain(
                    name=f"{ins.name}-presync{i}", ins=[], outs=[],
                    bass_is_fusable=False,
                )
                nd.engine = ins.engine
                nd.sync_info = _br.SyncInfo(on_wait=[w], on_update=[])
                nc.register_instruction(nd, overwrite=True)
                bb.instructions.insert(pos, nd)
                pos += 1
            si.on_wait = keep
            ins.sync_info = si
    return nc


_NC_CACHE = {}


def _get_nc():
    if "nc" not in _NC_CACHE:
        _NC_CACHE["nc"] = _build()
    return _NC_CACHE["nc"]


def kernel(logits, labels):
    global LAST_RESULTS
    logits = np.ascontiguousarray(np.asarray(logits), dtype=np.float32)
    labels_i = np.asarray(labels).astype(np.int64)
    assert logits.shape == (N, C), logits.shape

    # host-side gather of the label logit (1% of input bytes)
    gvals = logits[np.arange(N), labels_i].astype(np.float32)

    # bias constants: -c'_b for the conf relu/sign passes, then -(c'_b - 2)
    cnrow = np.array(
        [-np.float32(t) for t in THRP] + [-np.float32(t) for t in THRU],
        dtype=np.float32,
    )
    cnarr = np.ascontiguousarray(np.broadcast_to(cnrow, (P, 32)))

    in_maps = []
    for c in range(NCORES):
        lo, hi = c * ROWS, (c + 1) * ROWS
        if hi <= N:
            xs = logits[lo:hi]
            gc = gvals[lo:hi]
        else:
            xs = np.concatenate(
                [logits[lo:], np.zeros((hi - N, C), np.float32)], axis=0
            )
            gc = np.concatenate([gvals[lo:], np.zeros(hi - N, np.float32)])
        in_maps.append(
            {
                "x": np.ascontiguousarray(xs.reshape(P, SPP * C)),
                "g": np.ascontiguousarray(gc.reshape(P, SPP)),
                "cn": cnarr,
            }
        )

    trace = bool(int(os.environ.get("ECE_TRACE", "0")))
    res = run_bass_kernel_spmd(
        _get_nc(), in_maps, core_ids=list(range(NCORES)), trace=trace
    )
    LAST_RESULTS = res

    z16 = lambda: np.zeros(NBINS, np.float64)
    R0, sgc0, sgu0 = z16(), z16(), z16()   # chunk 0: pure ACT scheme
    sumconf0 = 0.0
    sgc1, sgu1, wt1 = z16(), z16(), z16()  # chunk 1: ACT signs + DVE wt
    wt2, nn2, av2 = z16(), z16(), z16()    # chunk 2: pure DVE scheme
    for out in res.results:
        ob = out["bins"].astype(np.float64)
        R0 += ob[:, 0:16].sum(axis=0)
        sgc0 += ob[:, 16:32].sum(axis=0)
        sgu0 += ob[:, 32:48].sum(axis=0)
        sumconf0 += ob[:, 48].sum()
        sgc1 += ob[:, 49 + 16 : 49 + 32].sum(axis=0)
        sgu1 += ob[:, 49 + 32 : 49 + 48].sum(axis=0)
        wt2 += ob[:, 98 : 98 + 16].sum(axis=0)
        nn2 += ob[:, 98 + 16 : 98 + 32].sum(axis=0)
        av2 += ob[:, 98 + 32 : 98 + 48].sum(axis=0)
        wt1 += ob[:, 98 + 49 : 98 + 65].sum(axis=0)

    # per-chunk sample totals (positional; pads included)
    L = [(hi - lo) * P * NCORES for lo, hi in CHUNKS]
    thrp64 = np.array([np.float64(t) for t in THRP])
    thrb64 = np.array([np.float64(t) for t in THRB])

    nn0 = (L[0] - sgc0) / 2.0             # {conf <= c_b}, exact counts
    A0 = (L[0] - sgu0) / 2.0              # {acc=1 and conf <= c_b}
    S0 = sumconf0 - R0 - thrp64 * (L[0] - nn0)

    nn1 = (L[1] - sgc1) / 2.0
    A1 = (L[1] - sgu1) / 2.0
    S1 = wt1 - thrb64 * (L[1] - nn1)

    S2 = wt2 - thrb64 * (L[2] - nn2)

    # cumulative totals; then remove the PAD rows (conf_bf = bf16(0.01) <=
    # every threshold, acc=1)
    S = S0 + S1 + S2 - PAD * PAD_CONF_BF
    A = A0 + A1 + av2 - PAD
    conf_sum = np.diff(S, prepend=0.0)
    acc_sum = np.diff(A, prepend=0.0)
    ece = np.abs(conf_sum - acc_sum).sum() / N
    return np.array([ece], dtype=np.float32)


# revision 48
# speedup vs baseline: 1.1245x; 1.0213x over previous
"""ECE loss kernel for Trainium2 (Bass/Tile), data-parallel over 8 NeuronCores.

Math (per sample row of logits[N, C]):
  conf = max softmax(x) = exp(max(x)) / sum(exp(x))
  acc  = (argmax(x) == label)
  ece  = sum_b |conf_sum[b] - acc_sum[b]| / N     (15 bins + empty tail)

Device pipeline per core (125184 rows as [128 partitions x 978 samples],
tiles of 32 samples x 100 classes):
  - DMA x tiles on the two HWDGE rings (sync/scalar)
  - ACT: E = exp(x) in place
  - DVE: reduce_max + reduce_sum over E per tile (the irreducible 1x passes;
    ~85% of DVE time)
  - binning runs incrementally in 3 sample-chunks so it overlaps the main
    loop.  The first two chunks bin on the ACT engine via accumulate:
      wt-side: sum relu(conf - c') and sum conf  (min(conf,c) identity)
      counts:  sum sign(conf - c') with c' strictly between two bf16 values,
               so sign is exactly +-1 and (L - sum)/2 is an exact count
      acc:     same sign trick on u = conf - 2*acc
    conf is bf16-rounded once (all passes see the same values, so per-sample
    bin assignment is consistent; boundary nudges cancel to ~1e-6 in ECE).
    The small tail chunk bins on the then-idle DVE (min/is_le + accum).
Host: gathers g = x[i,label_i] (1% of bytes), pads 1472 zero rows, merges the
per-core [128, 147] outputs and finishes the tiny 16-bin reduction.
"""

import os

import numpy as np

import concourse.bass as bass
import concourse.mybir as mybir
import concourse.tile as tile
from concourse.bass_utils import run_bass_kernel_spmd

F32 = mybir.dt.float32
ALU = mybir.AluOpType
AX = mybir.AxisListType
ACTF = mybir.ActivationFunctionType

N = 1_000_000
C = 100
NCORES = 8
P = 128
SPP = 978                   # samples per partition (padded)
ROWS = P * SPP              # 125184 rows per core
NTOT = NCORES * ROWS        # 1001472
PAD = NTOT - N              # 1472 zero pad rows (conf=0.01, acc=1)
K = 32                      # samples per tile
SIZES = [16, 16] + [K] * 29 + [18]   # sum = 978; small leading tiles so the
                                     # first reduces start sooner
CHUNKS = [(0, 512), (512, 896), (896, 978)]  # binning chunks (tile-aligned)
N_ACT_CHUNKS = 2            # first chunks bin on ACT (overlap the main loop);
                            # the small tail chunk bins on the then-idle DVE
NBINS = 16

LAST_RESULTS = None


def _bin_thresholds():
    """C_b = largest f32 y such that f32(15*y) <= b+1, for b = 0..14."""
    thr = []
    for b in range(15):
        tgt = np.float32(b + 1)

        def f(v):
            return np.float32(np.float32(15.0) * v)

        y = np.float32((b + 1) / 15.0)
        if f(y) <= tgt:
            while True:
                y2 = np.nextafter(y, np.float32(np.inf))
                if f(y2) <= tgt:
                    y = y2
                else:
                    break
        else:
            while f(y) > tgt:
                y = np.nextafter(y, np.float32(-np.inf))
        thr.append(np.float32(y))
    thr.append(np.float32(1e9))  # catch-all last segment
    return thr


THR = _bin_thresholds()


def _to_bf16(x):
    """Round f32 -> nearest-even bf16, returned as exactly-representable f32."""
    u = int(np.float32(x).view(np.uint32))
    upper, lower = u >> 16, u & 0xFFFF
    if lower > 0x8000 or (lower == 0x8000 and (upper & 1)):
        upper += 1
    return np.uint32(upper << 16).view(np.float32)


# bf16-representable bin boundaries: the device bins the bf16-rounded conf
# against these, and the host S_b identity uses the same exact values
THRB = [_to_bf16(t) for t in THR[:15]] + [np.float32(2.0 ** 30)]
PAD_CONF_BF = float(_to_bf16(np.float32(np.float32(1.0) * np.float32(1.0 / np.float32(100.0)))))


def _next_bf16_up(x):
    u = int(np.float32(x).view(np.uint32))
    return np.uint32(((u >> 16) + 1) << 16).view(np.float32)


# off-grid thresholds strictly between consecutive bf16 values: a bf16 conf
# can never equal one, so sign(conf - c') is exactly +-1 and
# (N - sum sign)/2 counts {conf <= c_b} exactly
THRP = [
    np.float32(
        np.float64(THRB[b]) + (np.float64(_next_bf16_up(THRB[b])) - np.float64(THRB[b])) / 2
    )
    for b in range(15)
] + [np.float32(1.5 * 2.0 ** 30)]
# u = conf - 2*acc thresholds: {u <= c'-2} == {acc=1 and conf <= c_b}
THRU = [np.float32(np.float32(t) - np.float32(2.0)) for t in THRP[:15]] + [
    np.float32(-0.5)
]


def _build():
    nc = bass.Bass(trn_type="TRN2")
    x = nc.dram_tensor("x", [P, SPP * C], F32, kind="ExternalInput")
    g = nc.dram_tensor("g", [P, SPP], F32, kind="ExternalInput")
    cn = nc.dram_tensor("cn", [P, 32], F32, kind="ExternalInput")
    bins = nc.dram_tensor(
        "bins", [P, 49 * len(CHUNKS) + 16], F32, kind="ExternalOutput"
    )

    with tile.TileContext(nc) as tc:
        BF16 = mybir.dt.bfloat16
        with (
            tc.tile_pool(name="xin", bufs=6) as xin,
            tc.tile_pool(name="persist", bufs=1) as persist,
        ):
            g_sb = persist.tile([P, SPP], F32)
            em = persist.tile([P, SPP], F32)
            s_t = persist.tile([P, SPP], F32)
            rs = persist.tile([P, SPP], F32)
            conf_bf = persist.tile([P, SPP], BF16)
            acc_bf = persist.tile([P, SPP], BF16)
            junk_bf = persist.tile([P, SPP], BF16)
            u_t = persist.tile([P, SPP], F32)       # u = conf - 2*acc
            junk_act = persist.tile([P, SPP], F32)  # ACT binning discard out
            # separate accumulator tiles per engine so the DVE tail binning
            # never serializes behind the ACT chunks' accumulate writes
            bins_a = persist.tile([P, 49 * N_ACT_CHUNKS], F32)
            # 49 cols for the DVE tail chunk + 16 for chunk-1's wt sums
            # (its relu ops run on DVE so ACT's final burst is sign-only)
            bins_d = persist.tile([P, 49 * (len(CHUNKS) - N_ACT_CHUNKS) + 16], F32)
            cn_sb = persist.tile([P, 32], F32)
            nc.gpsimd.dma_start(out=g_sb[:, :], in_=g[:, :])
            nc.gpsimd.dma_start(out=cn_sb[:, :], in_=cn[:, :])

            def emit_chunk(ci, lo, hi):
                sl = slice(lo, hi)
                if ci < N_ACT_CHUNKS:
                    bins_sb, base = bins_a, 49 * ci
                else:
                    bins_sb, base = bins_d, 49 * (ci - N_ACT_CHUNKS)
                nc.vector.reciprocal(rs[:, sl], s_t[:, sl])
                # conf_bf = bf16(em / s): all three per-bin sums see the SAME
                # rounded values, so per-sample bin assignment is consistent;
                # rounding only nudges boundary samples between adjacent bins
                nc.vector.tensor_mul(conf_bf[:, sl], em[:, sl], rs[:, sl])
                nc.vector.tensor_tensor(
                    acc_bf[:, sl], g_sb[:, sl], em[:, sl], op=ALU.is_equal
                )
                if ci < N_ACT_CHUNKS:
                    # u = conf - 2*acc  (exact in fp32)
                    nc.vector.scalar_tensor_tensor(
                        u_t[:, sl], acc_bf[:, sl], -2.0, conf_bf[:, sl],
                        op0=ALU.mult, op1=ALU.add,
                    )
                junk = junk_act
                if ci < N_ACT_CHUNKS:
                    if ci < N_ACT_CHUNKS - 1:
                        nc.scalar.activation(
                            junk[:, sl], conf_bf[:, sl], ACTF.Copy,
                            accum_out=bins_sb[:, base + 48 : base + 49],
                        )
                        for b in range(NBINS):
                            nc.scalar.activation(
                                junk[:, sl], conf_bf[:, sl], ACTF.Relu,
                                bias=cn_sb[:, b : b + 1],
                                accum_out=bins_sb[:, base + b : base + b + 1],
                            )
                    else:
                        # last ACT chunk: wt via DVE min-trick in the tail
                        # window, so ACT's closing burst is sign-ops only
                        for b in range(NBINS):
                            nc.vector.tensor_scalar(
                                junk_bf[:, sl], conf_bf[:, sl],
                                float(THRB[b]), None,
                                op0=ALU.min, op1=ALU.add,
                                accum_out=bins_d[:, 49 + b : 50 + b],
                            )
                    for b in range(NBINS):
                        nc.scalar.activation(
                            junk[:, sl], conf_bf[:, sl], ACTF.Sign,
                            bias=cn_sb[:, b : b + 1],
                            accum_out=bins_sb[:, base + 16 + b : base + 17 + b],
                        )
                    for b in range(NBINS):
                        nc.scalar.activation(
                            junk[:, sl], u_t[:, sl], ACTF.Sign,
                            bias=cn_sb[:, 16 + b : 17 + b],
                            accum_out=bins_sb[:, base + 32 + b : base + 33 + b],
                        )
                else:
                    # tail chunk: DVE min/count binning (DVE is idle here)
                    for b in range(NBINS):
                        cb = float(THRB[b])
                        nc.vector.tensor_scalar(
                            junk_bf[:, sl], conf_bf[:, sl], cb, None,
                            op0=ALU.min, op1=ALU.add,
                            accum_out=bins_sb[:, base + b : base + b + 1],
                        )
                        nc.vector.tensor_scalar(
                            junk_bf[:, sl], conf_bf[:, sl], cb, None,
                            op0=ALU.is_le, op1=ALU.add,
                            accum_out=bins_sb[:, base + 16 + b : base + 17 + b],
                        )
                        nc.vector.scalar_tensor_tensor(
                            junk_bf[:, sl], conf_bf[:, sl], cb, acc_bf[:, sl],
                            op0=ALU.is_le, op1=ALU.mult,
                            accum_out=bins_sb[:, base + 32 + b : base + 33 + b],
                        )

            dma_engines = [nc.sync, nc.scalar]
            off = 0
            ci = 0
            for t, k in enumerate(SIZES):
                kc = k * C
                xt = xin.tile([P, K * C], F32, tag="xt")
                dma_engines[t % 2].dma_start(
                    out=xt[:, :kc], in_=x[:, off * C : (off + k) * C]
                )
                nc.scalar.activation(xt[:, :kc], xt[:, :kc], ACTF.Exp)
                ev = xt[:, :kc].rearrange("p (k c) -> p k c", c=C)
                nc.vector.reduce_max(
                    out=em[:, off : off + k], in_=ev[:, :, :], axis=AX.X
                )
                nc.vector.reduce_sum(
                    out=s_t[:, off : off + k], in_=ev[:, :, :], axis=AX.X
                )
                off += k
                if t == 3:
                    # exp(g) is tile-independent: run it early (ahead of every
                    # chunk's accuracy compare and ACT's binning bursts) but
                    # NOT first in ACT's in-order queue, where it would gate
                    # tile-0's exp behind the g-load DMA
                    nc.scalar.activation(g_sb[:, :], g_sb[:, :], ACTF.Exp)
                if ci < len(CHUNKS) and CHUNKS[ci][1] == off:
                    # high priority: run the chunk's conf/acc ops ahead of
                    # later tiles' reduces so the ACT binning burst starts
                    # as early as its data allows
                    with tc.high_priority():
                        emit_chunk(ci, CHUNKS[ci][0], CHUNKS[ci][1])
                    ci += 1
            assert ci == len(CHUNKS)
            nc.sync.dma_start(out=bins[:, 0 : 49 * N_ACT_CHUNKS], in_=bins_a[:, :])
            nc.sync.dma_start(out=bins[:, 49 * N_ACT_CHUNKS :], in_=bins_d[:, :])

    # ---- sync-command budget fixes (instructions carry <= 2 sync commands:
    # completion update + at most one wait).  Drop waits provably covered by
    # earlier waits on the same engine, then split any remaining multi-wait
    # instruction across preceding same-engine drains.
    import re as _re

    def _tick_sem(name):
        return bool(_re.match(
            r"^(Activation|DVE|PE|Pool|SP|DMAHW\d+|DMASW\d+)_\d+$", name
        ))

    seen_waits = {}
    for bb in nc.m.functions[0].blocks:
        for ins in bb.instructions:
            si = ins.sync_info
            if si is None:
                continue
            tname = type(ins).__name__
            if tname == "InstEventSemaphore":
                continue
            eng = str(ins.engine).split(".")[-1]
            kept = list(si.on_wait)
            if tname not in ("InstDMACopy", "InstDrain") and len(kept) > 1:
                # same-engine waits are redundant (program order)
                kept = [w for w in kept if not w.ant_name.startswith(f"{eng}_")]
            kept2 = []
            for w in kept:
                if not _tick_sem(w.ant_name):
                    kept2.append(w)
                elif seen_waits.get((eng, w.ant_name), -1) < w.wait_value:
                    kept2.append(w)
            kept = kept2
            for w in kept:
                if not _tick_sem(w.ant_name):
                    continue
                key = (eng, w.ant_name)
                seen_waits[key] = max(seen_waits.get(key, -1), w.wait_value)
            if len(kept) != len(si.on_wait):
                si.on_wait = kept
                ins.sync_info = si

    import bass_rust as _br

    for bb in nc.m.functions[0].blocks:
        while True:
            insns = list(bb.instructions)
            target = None
            for idx, ins in enumerate(insns):
                si = ins.sync_info
                if si is None:
                    continue
                if len(si.on_wait) > 1:
                    target = (idx, ins)
                    break
            if target is None:
                break
            idx, ins = target
            si = ins.sync_info
            waits = list(si.on_wait)
            if type(ins).__name__ == "InstDrain":
                room = max(0, 1 - len(si.on_update))
            else:
                room = 1
            keep, extra = waits[len(waits) - room :], waits[: len(waits) - room]
            pos = idx
            for i, w in enumerate(extra):
                nd = mybir.InstDrain(
                    name=f"{ins.name}-presync{i}", ins=[], outs=[],
                    bass_is_fusable=False,
                )
                nd.engine = ins.engine
                nd.sync_info = _br.SyncInfo(on_wait=[w], on_update=[])
                nc.register_instruction(nd, overwrite=True)
                bb.instructions.insert(pos, nd)
                pos += 1
            si.on_wait = keep
            ins.sync_info = si
    return nc


_NC_CACHE = {}


def _get_nc():
    if "nc" not in _NC_CACHE:
        _NC_CACHE["nc"] = _build()
    return _NC_CACHE["nc"]


def kernel(logits, labels):
    global LAST_RESULTS
    logits = np.ascontiguousarray(np.asarray(logits), dtype=np.float32)
    labels_i = np.asarray(labels).astype(np.int64)
    assert logits.shape == (N, C), logits.shape

    # host-side gather of the label logit (1% of input bytes)
    gvals = logits[np.arange(N), labels_i].astype(np.float32)

    # bias constants: -c'_b for the conf relu/sign passes, then -(c'_b - 2)
    cnrow = np.array(
        [-np.float32(t) for t in THRP] + [-np.float32(t) for t in THRU],
        dtype=np.float32,
    )
    cnarr = np.ascontiguousarray(np.broadcast_to(cnrow, (P, 32)))

    in_maps = []
    for c in range(NCORES):
        lo, hi = c * ROWS, (c + 1) * ROWS
        if hi <= N:
            xs = logits[lo:hi]
            gc = gvals[lo:hi]
        else:
            xs = np.concatenate(
                [logits[lo:], np.zeros((hi - N, C), np.float32)], axis=0
            )
            gc = np.concatenate([gvals[lo:], np.zeros(hi - N, np.float32)])
        in_maps.append(
            {
                "x": np.ascontiguousarray(xs.reshape(P, SPP * C)),
                "g": np.ascontiguousarray(gc.reshape(P, SPP)),
                "cn": cnarr,
            }
        )

    trace = bool(int(os.environ.get("ECE_TRACE", "0")))
    res = run_bass_kernel_spmd(
        _get_nc(), in_maps, core_ids=list(range(NCORES)), trace=trace
    )
    LAST_RESULTS = res

    z16 = lambda: np.zeros(NBINS, np.float64)
    R0, sgc0, sgu0 = z16(), z16(), z16()   # chunk 0: pure ACT scheme
    sumconf0 = 0.0
    sgc1, sgu1, wt1 = z16(), z16(), z16()  # chunk 1: ACT signs + DVE wt
    wt2, nn2, av2 = z16(), z16(), z16()    # chunk 2: pure DVE scheme
    for out in res.results:
        ob = out["bins"].astype(np.float64)
        R0 += ob[:, 0:16].sum(axis=0)
        sgc0 += ob[:, 16:32].sum(axis=0)
        sgu0 += ob[:, 32:48].sum(axis=0)
        sumconf0 += ob[:, 48].sum()
        sgc1 += ob[:, 49 + 16 : 49 + 32].sum(axis=0)
        sgu1 += ob[:, 49 + 32 : 49 + 48].sum(axis=0)
        wt2 += ob[:, 98 : 98 + 16].sum(axis=0)
        nn2 += ob[:, 98 + 16 : 98 + 32].sum(axis=0)
        av2 += ob[:, 98 + 32 : 98 + 48].sum(axis=0)
        wt1 += ob[:, 98 + 49 : 98 + 65].sum(axis=0)

    # per-chunk sample totals (positional; pads included)
    L = [(hi - lo) * P * NCORES for lo, hi in CHUNKS]
    thrp64 = np.array([np.float64(t) for t in THRP])
    thrb64 = np.array([np.float64(t) for t in THRB])

    nn0 = (L[0] - sgc0) / 2.0             # {conf <= c_b}, exact counts
    A0 = (L[0] - sgu0) / 2.0              # {acc=1 and conf <= c_b}
    S0 = sumconf0 - R0 - thrp64 * (L[0] - nn0)

    nn1 = (L[1] - sgc1) / 2.0
    A1 = (L[1] - sgu1) / 2.0
    S1 = wt1 - thrb64 * (L[1] - nn1)

    S2 = wt2 - thrb64 * (L[2] - nn2)

    # cumulative totals; then remove the PAD rows (conf_bf = bf16(0.01) <=
    # every threshold, acc=1)
    S = S0 + S1 + S2 - PAD * PAD_CONF_BF
    A = A0 + A1 + av2 - PAD
    conf_sum = np.diff(S, prepend=0.0)
    acc_sum = np.diff(A, prepend=0.0)
    ece = np.abs(conf_sum - acc_sum).sum() / N
    return np.array([ece], dtype=np.float32)
